# revision 13
# baseline (speedup 1.0000x reference)
"""BWGNN (beta-wavelet GNN) forward on 8 TRN2 NeuronCores.

Sharding: nodes row-sharded 12500/core (padded to 12544 = 98 blocks of 128);
edges partitioned by dst so scatter-adds are local; per hop the bf16 feature
table is AllGathered and src rows are fetched by indirect DMA; the
segment-sum is a one-hot matmul into PSUM per 128-node dst block. Dense
weights replicated; the beta-wavelet polynomial is folded into the MLP-head
weights so only h, Lh, L^2h are needed.

kernel(**inputs) -> np.ndarray [100000, 2] float32
"""

import math
from contextlib import ExitStack

import numpy as np
import ml_dtypes

import concourse.bacc as bacc
import concourse.bass as bass
import concourse.mybir as mybir
from concourse.bass_utils import run_bass_kernel_spmd

# ---- problem constants (hardcoded) ----
N_NODES = 100000
N_EDGES = 1600000
FIN = 512
H = 128
D_POLY = 2
NCORES = 8
P = 128
PC = N_NODES // NCORES            # 12500
NB = (PC + P - 1) // P            # 98
PCP = NB * P                      # 12544
NPAD = NCORES * PCP               # 100352
CH = 512                          # node cols per dense chunk
NCH = (PCP + CH - 1) // CH        # 25 (last chunk 256)

f32 = mybir.dt.float32
bf16 = mybir.dt.bfloat16
i32 = mybir.dt.int32

AF = mybir.ActivationFunctionType
ALU = mybir.AluOpType


def _calculate_theta(d):
    thetas = []
    for i in range(d + 1):
        p1 = np.zeros(i + 1)
        p1[i] = 0.5 ** i
        m = d - i
        p2 = np.array([math.comb(m, j) * (-0.5) ** j for j in range(m + 1)])
        c = np.convolve(p1, p2)
        beta = math.gamma(i + 1) * math.gamma(d + 1 - i) / math.gamma(d + 2)
        thetas.append(c / beta)
    return thetas


def build(tpb, stage=5):
    """Build the SPMD bass program. tpb = edge tiles per dst block."""
    nc = bacc.Bacc("TRN2", target_bir_lowering=False, debug=False,
                   num_devices=NCORES)

    # ---- DRAM I/O ----
    xT = nc.dram_tensor("xT", [FIN, PCP], bf16, kind="ExternalInput")
    w1 = nc.dram_tensor("w1", [FIN, H], bf16, kind="ExternalInput")
    w2 = nc.dram_tensor("w2", [H, H], bf16, kind="ExternalInput")
    c0 = nc.dram_tensor("c0", [H, H], f32, kind="ExternalInput")
    c1 = nc.dram_tensor("c1", [H, H], f32, kind="ExternalInput")
    c2 = nc.dram_tensor("c2", [H, H], f32, kind="ExternalInput")
    wm2 = nc.dram_tensor("wm2", [H, 2], bf16, kind="ExternalInput")
    b1 = nc.dram_tensor("b1", [P, 1], f32, kind="ExternalInput")
    b2 = nc.dram_tensor("b2", [P, 1], f32, kind="ExternalInput")
    bm1 = nc.dram_tensor("bm1", [P, 1], f32, kind="ExternalInput")
    bm2 = nc.dram_tensor("bm2", [2, 1], f32, kind="ExternalInput")
    iota_in = nc.dram_tensor("iota_in", [P, P], bf16, kind="ExternalInput")
    ident_in = nc.dram_tensor("ident_in", [P, P], f32, kind="ExternalInput")
    idx_g = nc.dram_tensor("idx_g", [NB, P, tpb], i32, kind="ExternalInput")
    cw = nc.dram_tensor("cw", [NB, P, tpb], bf16, kind="ExternalInput")
    wv = nc.dram_tensor("wv", [NB, P, tpb], f32, kind="ExternalInput")
    outT = nc.dram_tensor("outT", [2, PCP], f32, kind="ExternalOutput")

    # internal DRAM
    h_local = nc.dram_tensor("h_local", [PCP, H], bf16)
    h_full = nc.dram_tensor("h_full", [NPAD, H], bf16)
    l_local = nc.dram_tensor("l_local", [PCP, H], bf16)
    l_full = nc.dram_tensor("l_full", [NPAD, H], bf16)

    chunk_cols = [min(CH, PCP - c * CH) for c in range(NCH)]
    N_WT = 15   # weight/constant DMAs

    with ExitStack() as ctx:
        ent = ctx.enter_context
        # ---- SBUF ----
        hT = ent(nc.sbuf_tensor("hT", [P, PCP], f32))        # h; later z_acc
        lhT = ent(nc.sbuf_tensor("lhT", [P, PCP], f32))
        l2hT = ent(nc.sbuf_tensor("l2hT", [P, PCP], f32))
        xt_sb = ent(nc.sbuf_tensor("xt_sb", [P, 2, 4, CH], bf16))
        h1_sb = ent(nc.sbuf_tensor("h1_sb", [P, 2, CH], bf16))
        w1sb = ent(nc.sbuf_tensor("w1sb", [P, 4, H], bf16))
        w2sb = ent(nc.sbuf_tensor("w2sb", [P, H], bf16))
        c0sb = ent(nc.sbuf_tensor("c0sb", [P, H], f32))
        c1sb = ent(nc.sbuf_tensor("c1sb", [P, H], f32))
        c2sb = ent(nc.sbuf_tensor("c2sb", [P, H], f32))
        wm2sb = ent(nc.sbuf_tensor("wm2sb", [P, 2], bf16))
        b1sb = ent(nc.sbuf_tensor("b1sb", [P, 1], f32))
        b2sb = ent(nc.sbuf_tensor("b2sb", [P, 1], f32))
        bm1sb = ent(nc.sbuf_tensor("bm1sb", [P, 1], f32))
        bm2sb = ent(nc.sbuf_tensor("bm2sb", [2, 1], f32))
        iota_sb = ent(nc.sbuf_tensor("iota_sb", [P, P], bf16))
        ident_sb = ent(nc.sbuf_tensor("ident_sb", [P, P], f32))
        g_sb = ent(nc.sbuf_tensor("g_sb", [P, 2, tpb, H], bf16))
        s_sb = ent(nc.sbuf_tensor("s_sb", [P, 2, tpb, H], bf16))
        sel_sb = ent(nc.sbuf_tensor("sel_sb", [P, 2, P], bf16))
        cw_sb = ent(nc.sbuf_tensor("cw_sb", [P, 2, tpb], bf16))
        wv_sb = ent(nc.sbuf_tensor("wv_sb", [P, 2, tpb], f32))
        idx_sb = ent(nc.sbuf_tensor("idx_sb", [P, 2, tpb], i32))
        nm_sb = ent(nc.sbuf_tensor("nm_sb", [P, 2, H], bf16))
        zt_sb = ent(nc.sbuf_tensor("zt_sb", [P, 2, CH], f32))
        zb_sb = ent(nc.sbuf_tensor("zb_sb", [P, 2, CH], bf16))
        ot_sb = ent(nc.sbuf_tensor("ot_sb", [2, 2, CH], f32))

        # PSUM: 8 banks of [128, 512] f32; three double-buffered tensors
        ps_a = ent(nc.psum_tensor("ps_a", [P, 2, CH], f32))  # mm1 / z passes
        ps_b = ent(nc.psum_tensor("ps_b", [P, 2, CH], f32))  # mm2 / aggs / out
        ps_t = ent(nc.psum_tensor("ps_t", [P, 2, CH], f32))  # transposes (bank-padded)

        # semaphores
        s_wt = ent(nc.semaphore("s_wt"))
        s_x = ent(nc.semaphore("s_x"))
        s_idx = ent(nc.semaphore("s_idx"))
        s_hl = ent(nc.semaphore("s_hl"))
        s_g = ent(nc.semaphore("s_g"))
        s_g2 = ent(nc.semaphore("s_g2"))
        s_out = ent(nc.semaphore("s_out"))
        cc_sem = ent(nc.semaphore("cc_sem"))
        pe1 = ent(nc.semaphore("pe1"))
        pe2 = ent(nc.semaphore("pe2"))
        petr = ent(nc.semaphore("petr"))
        pemm = ent(nc.semaphore("pemm"))
        pez = ent(nc.semaphore("pez"))
        peo = ent(nc.semaphore("peo"))
        act1 = ent(nc.semaphore("act1"))
        act2 = ent(nc.semaphore("act2"))
        actz = ent(nc.semaphore("actz"))
        arelu = ent(nc.semaphore("arelu"))
        abias = ent(nc.semaphore("abias"))
        dsub = ent(nc.semaphore("dsub"))
        dnm = ent(nc.semaphore("dnm"))
        dsel = ent(nc.semaphore("dsel"))
        dzadd = ent(nc.semaphore("dzadd"))
        block = ent(nc.Block())

        # ---------------- SYNC: HWDGE loads/stores ----------------
        @block.sync
        def _(sp):
            nwt = 0
            for k in range(4):
                sp.wait_ge(s_wt, 16 * nwt)
                sp.dma_start(w1sb[:, k], w1.ap()[k * P:(k + 1) * P, :]).then_inc(s_wt, 16)
                nwt += 1
            for dst_t, src_t in [(w2sb, w2), (c0sb, c0), (c1sb, c1), (c2sb, c2),
                                 (wm2sb, wm2), (b1sb, b1), (b2sb, b2),
                                 (bm1sb, bm1), (bm2sb, bm2),
                                 (iota_sb, iota_in), (ident_sb, ident_in)]:
                sp.wait_ge(s_wt, 16 * nwt)
                sp.dma_start(dst_t[:], src_t.ap()[:, :]).then_inc(s_wt, 16)
                nwt += 1

            # phase 1: x loads + h_local stores (store lags one chunk)
            nmi = 0
            for c in range(NCH):
                ncols = chunk_cols[c]
                if c >= 2:
                    sp.wait_ge(pe1, c - 1)
                if c >= 1:
                    sp.wait_ge(s_x, 64 * c)
                for k in range(4):
                    sp.dma_start(
                        xt_sb[:, c % 2, k, :ncols],
                        xT.ap()[k * P:(k + 1) * P, c * CH:c * CH + ncols],
                    ).then_inc(s_x, 16)
                if c >= 3:
                    for j in range(chunk_cols[c - 3] // P):
                        blk = 4 * (c - 3) + j
                        nmi += 1
                        sp.wait_ge(dnm, nmi)
                        sp.wait_ge(s_hl, 16 * (nmi - 1))
                        sp.dma_start(
                            h_local.ap()[blk * P:(blk + 1) * P, :],
                            nm_sb[:, (nmi - 1) % 2],
                        ).then_inc(s_hl, 16)
            for cc2 in range(NCH - 3, NCH):
                for j in range(chunk_cols[cc2] // P):
                    blk = 4 * cc2 + j
                    nmi += 1
                    sp.wait_ge(dnm, nmi)
                    sp.wait_ge(s_hl, 16 * (nmi - 1))
                    sp.dma_start(
                        h_local.ap()[blk * P:(blk + 1) * P, :],
                        nm_sb[:, (nmi - 1) % 2],
                    ).then_inc(s_hl, 16)
            assert nmi == NB

            # hop1: idx/cw loads + l_local stores (lag 2)
            for b in range(NB if stage >= 3 else 0):
                if b >= 2:
                    sp.wait_ge(pemm, b - 1)
                if b >= 1:
                    sp.wait_ge(s_idx, 48 * b)
                sp.dma_start(idx_sb[:, b % 2], idx_g.ap()[b]).then_inc(s_idx, 16)
                sp.dma_start(cw_sb[:, b % 2], cw.ap()[b]).then_inc(s_idx, 16)
                sp.dma_start(wv_sb[:, b % 2], wv.ap()[b]).then_inc(s_idx, 16)
                if b >= 2:
                    bb = b - 2
                    sp.wait_ge(dnm, NB + bb + 1)
                    sp.wait_ge(s_hl, 16 * (NB + bb))
                    sp.dma_start(
                        l_local.ap()[bb * P:(bb + 1) * P, :],
                        nm_sb[:, bb % 2],
                    ).then_inc(s_hl, 16)
            for bb in range(NB - 2 if stage >= 3 else NB, NB):
                sp.wait_ge(dnm, NB + bb + 1)
                sp.wait_ge(s_hl, 16 * (NB + bb))
                sp.dma_start(
                    l_local.ap()[bb * P:(bb + 1) * P, :],
                    nm_sb[:, bb % 2],
                ).then_inc(s_hl, 16)

            # hop2: idx/cw loads
            for b in range(NB if stage >= 4 else 0):
                if b >= 2:
                    sp.wait_ge(pemm, NB + b - 1)
                sp.wait_ge(s_idx, 48 * (NB + b))
                sp.dma_start(idx_sb[:, b % 2], idx_g.ap()[b]).then_inc(s_idx, 16)
                sp.dma_start(cw_sb[:, b % 2], cw.ap()[b]).then_inc(s_idx, 16)
                sp.dma_start(wv_sb[:, b % 2], wv.ap()[b]).then_inc(s_idx, 16)

            # head: outT stores  (actz counts: z1 NCH, relu NCH, bias NCH)
            for c in range(NCH if stage >= 5 else 0):
                ncols = chunk_cols[c]
                sp.wait_ge(abias, c + 1)
                sp.wait_ge(s_out, 16 * c)
                sp.dma_start(
                    outT.ap()[:, c * CH:c * CH + ncols],
                    ot_sb[:, c % 2, :ncols],
                ).then_inc(s_out, 16)

        # ---------------- GPSIMD: collectives + gathers ----------------
        @block.gpsimd
        def _(pl: bass.BassGpSimd):
            if stage < 2:
                return
            pl.wait_ge(s_hl, 16 * NB)
            pl.collective_compute(
                "AllGather", ALU.bypass,
                replica_groups=[list(range(NCORES))],
                ins=[h_local.ap().opt()],
                outs=[h_full.ap().opt()],
            ).then_inc(cc_sem, 1)
            pl.wait_ge(cc_sem, 1)
            for b in range(NB if stage >= 3 else 0):
                pl.wait_ge(s_idx, 48 * (b + 1))
                if b >= 2:
                    pl.wait_ge(pemm, b - 1)
                if b >= 1:
                    pl.wait_ge(s_g, 16 * tpb * b)
                for t in range(tpb):
                    pl.indirect_dma_start(
                        out=g_sb[:, b % 2, t], out_offset=None,
                        in_=h_full.ap()[:, :],
                        in_offset=bass.IndirectOffsetOnAxis(
                            ap=idx_sb[:, b % 2, t:t + 1], axis=0),
                    ).then_inc(s_g, 16)
            if stage < 4:
                return
            pl.wait_ge(s_hl, 16 * 2 * NB)
            pl.collective_compute(
                "AllGather", ALU.bypass,
                replica_groups=[list(range(NCORES))],
                ins=[l_local.ap().opt()],
                outs=[l_full.ap().opt()],
            ).then_inc(cc_sem, 1)
            pl.wait_ge(cc_sem, 2)
            for b in range(NB):
                pl.wait_ge(s_idx, 48 * NB + 48 * (b + 1))
                if b >= 2:
                    pl.wait_ge(pemm, NB + b - 1)
                if b >= 1:
                    pl.wait_ge(s_g2, 16 * tpb * b)
                for t in range(tpb):
                    pl.indirect_dma_start(
                        out=g_sb[:, b % 2, t], out_offset=None,
                        in_=l_full.ap()[:, :],
                        in_offset=bass.IndirectOffsetOnAxis(
                            ap=idx_sb[:, b % 2, t:t + 1], axis=0),
                    ).then_inc(s_g2, 16)

        # ---------------- TENSOR (PE) ----------------
        @block.tensor
        def _(pe: bass.BassTensorEngine):
            pe.wait_ge(s_wt, 16 * N_WT)

            def mm1(c):
                ncols = chunk_cols[c]
                pe.wait_ge(s_x, 64 * (c + 1))
                if c >= 2:
                    pe.wait_ge(act1, c - 1)
                mm = None
                for k in range(4):
                    mm = pe.matmul(ps_a[:, c % 2, :ncols], lhsT=w1sb[:, k],
                                   rhs=xt_sb[:, c % 2, k, :ncols],
                                   start=(k == 0), stop=(k == 3))
                mm.then_inc(pe1, 1)

            def mm2(c):
                ncols = chunk_cols[c]
                pe.wait_ge(act1, c + 1)
                if c >= 2:
                    pe.wait_ge(act2, c - 1)
                pe.matmul(ps_b[:, c % 2, :ncols], lhsT=w2sb[:],
                          rhs=h1_sb[:, c % 2, :ncols],
                          start=True, stop=True).then_inc(pe2, 1)

            tr_n = [0]

            def tr_phase1(c):
                pe.wait_ge(act2, c + 1)
                for j in range(chunk_cols[c] // P):
                    blk = 4 * c + j
                    i = tr_n[0]
                    if i >= 2:
                        pe.wait_ge(dnm, i - 1)
                    pe.transpose(ps_t[:, i % 2, :P], hT[:, blk * P:(blk + 1) * P],
                                 ident_sb[:]).then_inc(petr, 1)
                    tr_n[0] += 1

            for c in range(NCH + 2):
                if c < NCH:
                    mm1(c)
                if 1 <= c <= NCH:
                    mm2(c - 1)
                if 2 <= c:
                    tr_phase1(c - 2)
            assert tr_n[0] == NB

            def hop_mms(hop, b):
                gi = hop * NB + b
                pe.wait_ge(s_g if hop == 0 else s_g2, 16 * tpb * (b + 1))
                pe.wait_ge(dsel, 2 * tpb * (gi + 1))
                if b >= 2:
                    pe.wait_ge(dsub, gi - 1)
                elif hop == 0:
                    pe.wait_ge(act2, NCH)      # ps_b handoff from phase 1
                else:
                    pe.wait_ge(dsub, NB)       # ps_b handoff from hop 1
                mm = None
                for t in range(tpb):
                    mm = pe.matmul(ps_b[:, b % 2, :P],
                                   lhsT=g_sb[:, b % 2, t],
                                   rhs=s_sb[:, b % 2, t],
                                   start=(t == 0), stop=(t == tpb - 1))
                mm.then_inc(pemm, 1)

            def tr_hop1(b):
                pe.wait_ge(dsub, b + 1)
                i = NB + b
                pe.wait_ge(dnm, i - 1)
                pe.transpose(ps_t[:, i % 2, :P], lhT[:, b * P:(b + 1) * P],
                             ident_sb[:]).then_inc(petr, 1)

            if stage < 3:
                return
            for b in range(NB + 1):
                if b < NB:
                    hop_mms(0, b)
                if b >= 1:
                    tr_hop1(b - 1)

            if stage < 4:
                return
            # z1: ps_a = C0^T @ hT chunks (hop1 consumed hT)
            pe.wait_ge(dsub, NB)
            for c in range(NCH):
                ncols = chunk_cols[c]
                if c >= 2:
                    pe.wait_ge(actz, c - 1)
                else:
                    pe.wait_ge(act1, NCH)      # ps_a handoff from phase 1
                pe.matmul(ps_a[:, c % 2, :ncols], lhsT=c0sb[:],
                          rhs=hT[:, c * CH:c * CH + ncols],
                          start=True, stop=True).then_inc(pez, 1)

            for b in range(NB):
                hop_mms(1, b)
            if stage < 5:
                return

            # z2: ps_a = C1^T @ lhT
            for c in range(NCH):
                ncols = chunk_cols[c]
                if c >= 2:
                    pe.wait_ge(dzadd, c - 1)
                else:
                    pe.wait_ge(actz, NCH)      # z1 fully consumed ps_a
                pe.matmul(ps_a[:, c % 2, :ncols], lhsT=c1sb[:],
                          rhs=lhT[:, c * CH:c * CH + ncols],
                          start=True, stop=True).then_inc(pez, 1)

            # z3 + head
            pe.wait_ge(dsub, 2 * NB)
            def z3mm(c):
                ncols = chunk_cols[c]
                if c >= 2:
                    pe.wait_ge(dzadd, NCH + c - 1)
                else:
                    pe.wait_ge(dzadd, NCH)
                pe.matmul(ps_a[:, c % 2, :ncols], lhsT=c2sb[:],
                          rhs=l2hT[:, c * CH:c * CH + ncols],
                          start=True, stop=True).then_inc(pez, 1)

            def outmm(c):
                ncols = chunk_cols[c]
                pe.wait_ge(arelu, c + 1)           # zb ready
                if c >= 2:
                    pe.wait_ge(abias, c - 1)       # ot consumed -> ps_b free
                else:
                    pe.wait_ge(dsub, 2 * NB)       # hop2 aggs consumed ps_b
                pe.matmul(ps_b[0:2, c % 2, :ncols], lhsT=wm2sb[:],
                          rhs=zb_sb[:, c % 2, :ncols],
                          start=True, stop=True).then_inc(peo, 1)

            for c in range(NCH):
                z3mm(c)
                if c >= 1:
                    outmm(c - 1)
            outmm(NCH - 1)

        # ---------------- VECTOR (DVE) ----------------
        @block.vector
        def _(dv: bass.BassVectorEngine):
            dv.wait_ge(s_wt, 16 * N_WT)
            # phase 1 node-major casts
            for i in range(NB):
                dv.wait_ge(petr, i + 1)
                if i >= 2:
                    dv.wait_ge(s_hl, 16 * (i - 1))
                dv.tensor_copy(out=nm_sb[:, i % 2], in_=ps_t[:, i % 2, :P]) \
                    .then_inc(dnm, 1)

            def hop_dve(hop, b):
                gi = hop * NB + b
                dv.wait_ge(s_idx, 48 * gi + 48)
                src = hT if hop == 0 else lhT
                dst = lhT if hop == 0 else l2hT
                for t in range(tpb):
                    dv.tensor_tensor(
                        out=sel_sb[:, t % 2],
                        in0=cw_sb[:, b % 2, t:t + 1].to_broadcast([P, P]),
                        in1=iota_sb[:],
                        op=ALU.is_equal,
                    ).then_inc(dsel, 1)
                    dv.drain()
                    dv.tensor_scalar(
                        out=s_sb[:, b % 2, t], in0=sel_sb[:, t % 2],
                        scalar1=wv_sb[:, b % 2, t:t + 1],
                        scalar2=None, op0=ALU.mult,
                    ).then_inc(dsel, 1)
                dv.wait_ge(pemm, gi + 1)
                dv.tensor_tensor(
                    out=dst[:, b * P:(b + 1) * P],
                    in0=src[:, b * P:(b + 1) * P],
                    in1=ps_b[:, b % 2, :P],
                    op=ALU.subtract,
                ).then_inc(dsub, 1)

            def nm_hop1(b):
                dv.wait_ge(petr, NB + b + 1)
                dv.wait_ge(s_hl, 16 * (NB + b - 1))
                dv.tensor_copy(out=nm_sb[:, b % 2], in_=ps_t[:, (NB + b) % 2, :P]) \
                    .then_inc(dnm, 1)

            if stage < 3:
                return
            for b in range(NB):
                hop_dve(0, b)
                if b >= 1:
                    nm_hop1(b - 1)
            nm_hop1(NB - 1)

            if stage < 4:
                return
            for b in range(NB):
                hop_dve(1, b)
            if stage < 5:
                return

            # z2 accumulate into z_acc (= hT region)
            for c in range(NCH):
                ncols = chunk_cols[c]
                dv.wait_ge(pez, NCH + c + 1)
                dv.wait_ge(actz, c + 1)        # z1 wrote hT chunk c
                dv.tensor_tensor(
                    out=hT[:, c * CH:c * CH + ncols],
                    in0=hT[:, c * CH:c * CH + ncols],
                    in1=ps_a[:, c % 2, :ncols],
                    op=ALU.add,
                ).then_inc(dzadd, 1)
            dv.drain()
            # z3 accumulate into zt tiles
            for c in range(NCH):
                ncols = chunk_cols[c]
                dv.wait_ge(pez, 2 * NCH + c + 1)
                if c >= 2:
                    dv.wait_ge(arelu, c - 1)        # zt slot consumed by relu
                dv.tensor_tensor(
                    out=zt_sb[:, c % 2, :ncols],
                    in0=hT[:, c * CH:c * CH + ncols],
                    in1=ps_a[:, c % 2, :ncols],
                    op=ALU.add,
                ).then_inc(dzadd, 1)

        # ---------------- SCALAR (ACT) ----------------
        @block.scalar
        def _(ac: bass.BassScalarEngine):
            ac.wait_ge(s_wt, 16 * N_WT)
            for c in range(NCH):
                ncols = chunk_cols[c]
                ac.wait_ge(pe1, c + 1)
                if c >= 2:
                    ac.wait_ge(pe2, c - 1)     # h1 slot consumed by mm2
                ac.activation(h1_sb[:, c % 2, :ncols], ps_a[:, c % 2, :ncols],
                              AF.Relu, bias=b1sb[:, 0:1], scale=1.0) \
                    .then_inc(act1, 1)
                ac.wait_ge(pe2, c + 1)
                ac.activation(hT[:, c * CH:c * CH + ncols],
                              ps_b[:, c % 2, :ncols],
                              AF.Relu, bias=b2sb[:, 0:1], scale=1.0) \
                    .then_inc(act2, 1)
            # z1: z_acc = ps_a + bm1 (in place over hT)
            if stage < 4:
                return
            for c in range(NCH):
                ncols = chunk_cols[c]
                ac.wait_ge(pez, c + 1)
                ac.activation(hT[:, c * CH:c * CH + ncols],
                              ps_a[:, c % 2, :ncols],
                              AF.Identity, bias=bm1sb[:, 0:1], scale=1.0) \
                    .then_inc(actz, 1)
            # z3 relu -> bf16, interleaved with out-bias
            def relu_c(c):
                ncols = chunk_cols[c]
                ac.wait_ge(dzadd, NCH + c + 1)
                if c >= 2:
                    ac.wait_ge(peo, c - 1)     # zb slot consumed by out mm
                ac.activation(zb_sb[:, c % 2, :ncols], zt_sb[:, c % 2, :ncols],
                              AF.Relu, bias=0.0, scale=1.0) \
                    .then_inc(arelu, 1)

            def bias_c(c):
                ncols = chunk_cols[c]
                ac.wait_ge(peo, c + 1)
                if c >= 2:
                    ac.wait_ge(s_out, 16 * (c - 1))
                ac.activation(ot_sb[:, c % 2, :ncols], ps_b[0:2, c % 2, :ncols],
                              AF.Identity, bias=bm2sb[:, 0:1], scale=1.0) \
                    .then_inc(abias, 1)

            if stage < 5:
                return
            for c in range(NCH):
                relu_c(c)
                if c >= 1:
                    bias_c(c - 1)
            bias_c(NCH - 1)

    nc.compile()
    return nc


def host_prep(x, src, dst, W1, b1, W2, b2, Wm1, bm1, Wm2, bm2):
    """Shard + preprocess inputs. Returns (in_maps, tpb)."""
    src = np.asarray(src).astype(np.int64)
    dst = np.asarray(dst).astype(np.int64)
    x = np.asarray(x, dtype=np.float32)

    thetas = _calculate_theta(D_POLY)
    C = [sum(float(thetas[i][k]) * np.asarray(Wm1, np.float64)[i * H:(i + 1) * H]
             for i in range(D_POLY + 1)).astype(np.float32)
         for k in range(D_POLY + 1)]
    C = [np.ascontiguousarray(ck) for ck in C]

    deg = np.bincount(dst, minlength=N_NODES)
    dinv = np.clip(deg, 1, None).astype(np.float64) ** -0.5
    w_edge = (dinv[dst] * dinv[src]).astype(np.float32)

    core_of = dst // PC
    loc = dst - core_of * PC
    blk = loc // P
    col = (loc % P).astype(np.float32)
    gsrc = ((src // PC) * PCP + (src % PC)).astype(np.int32)

    flat = core_of * NB + blk
    order = np.argsort(flat, kind="stable")
    counts = np.bincount(flat, minlength=NCORES * NB)
    cum = np.concatenate([[0], np.cumsum(counts)])
    tpb = int(np.ceil(counts.max() / P))

    iota_np = np.tile(
        np.arange(P, dtype=np.float32).astype(ml_dtypes.bfloat16)[None, :],
        (P, 1))
    ident_np = np.eye(P, dtype=np.float32)

    w1_b = np.ascontiguousarray(np.asarray(W1, np.float32)).astype(ml_dtypes.bfloat16)
    w2_b = np.ascontiguousarray(np.asarray(W2, np.float32)).astype(ml_dtypes.bfloat16)
    wm2_b = np.ascontiguousarray(np.asarray(Wm2, np.float32)).astype(ml_dtypes.bfloat16)

    in_maps = []
    for c in range(NCORES):
        xc = np.zeros((PCP, FIN), np.float32)
        xc[:PC] = x[c * PC:(c + 1) * PC]
        xT_c = np.ascontiguousarray(xc.T).astype(ml_dtypes.bfloat16)

        idx_gc = np.zeros((NB, P, tpb), np.int32)
        cw_c = np.zeros((NB, P, tpb), np.float32)
        wv_c = np.zeros((NB, P, tpb), np.float32)
        for b in range(NB):
            fi = c * NB + b
            es = order[cum[fi]:cum[fi + 1]]
            n = len(es)
            sl_idx = np.zeros(tpb * P, np.int32)
            sl_col = np.zeros(tpb * P, np.float32)
            sl_w = np.zeros(tpb * P, np.float32)
            sl_idx[:n] = gsrc[es]
            sl_col[:n] = col[es]
            sl_w[:n] = w_edge[es]
            idx_gc[b] = sl_idx.reshape(tpb, P).T
            cw_c[b] = sl_col.reshape(tpb, P).T
            wv_c[b] = sl_w.reshape(tpb, P).T

        in_maps.append(dict(
            xT=np.asarray(xT_c),
            w1=np.asarray(w1_b), w2=np.asarray(w2_b),
            c0=C[0], c1=C[1], c2=C[2],
            wm2=np.asarray(wm2_b),
            b1=np.ascontiguousarray(np.asarray(b1, np.float32).reshape(P, 1)),
            b2=np.ascontiguousarray(np.asarray(b2, np.float32).reshape(P, 1)),
            bm1=np.ascontiguousarray(np.asarray(bm1, np.float32).reshape(P, 1)),
            bm2=np.ascontiguousarray(np.asarray(bm2, np.float32).reshape(2, 1)),
            iota_in=np.asarray(iota_np),
            ident_in=ident_np,
            idx_g=idx_gc,
            cw=np.asarray(cw_c.astype(ml_dtypes.bfloat16)),
            wv=wv_c,
        ))
    return in_maps, tpb


_BUILD_CACHE = {}


def kernel(x, src, dst, W1, b1, W2, b2, Wm1, bm1, Wm2, bm2, _trace=False):
    in_maps, tpb = host_prep(x, src, dst, W1, b1, W2, b2, Wm1, bm1, Wm2, bm2)
    if tpb not in _BUILD_CACHE:
        _BUILD_CACHE[tpb] = build(tpb)
    nc = _BUILD_CACHE[tpb]
    r = run_bass_kernel_spmd(nc, in_maps, list(range(NCORES)), trace=_trace)
    out = np.zeros((N_NODES, 2), np.float32)
    for c in range(NCORES):
        out[c * PC:(c + 1) * PC] = np.asarray(r.results[c]["outT"]).T[:PC]
    kernel._last_results = r
    return out


# revision 14
# speedup vs baseline: 1.1549x; 1.1549x over previous
"""BWGNN (beta-wavelet GNN) forward on 8 TRN2 NeuronCores.

Sharding: nodes row-sharded 12500/core (padded to 12544 = 98 blocks of 128);
edges partitioned by dst so scatter-adds are local; per hop the bf16 feature
table is AllGathered and src rows are fetched by indirect DMA; the
segment-sum is a one-hot matmul into PSUM per 128-node dst block. Dense
weights replicated; the beta-wavelet polynomial is folded into the MLP-head
weights so only h, Lh, L^2h are needed.

kernel(**inputs) -> np.ndarray [100000, 2] float32
"""

import math
from contextlib import ExitStack

import numpy as np
import ml_dtypes

import concourse.bacc as bacc
import concourse.bass as bass
import concourse.mybir as mybir
from concourse.bass_utils import run_bass_kernel_spmd

# ---- problem constants (hardcoded) ----
N_NODES = 100000
N_EDGES = 1600000
FIN = 512
H = 128
D_POLY = 2
NCORES = 8
P = 128
PC = N_NODES // NCORES            # 12500
NB = (PC + P - 1) // P            # 98
PCP = NB * P                      # 12544
NPAD = NCORES * PCP               # 100352
CH = 512                          # node cols per dense chunk
NCH = (PCP + CH - 1) // CH        # 25 (last chunk 256)

f32 = mybir.dt.float32
bf16 = mybir.dt.bfloat16
i32 = mybir.dt.int32

AF = mybir.ActivationFunctionType
ALU = mybir.AluOpType


def _calculate_theta(d):
    thetas = []
    for i in range(d + 1):
        p1 = np.zeros(i + 1)
        p1[i] = 0.5 ** i
        m = d - i
        p2 = np.array([math.comb(m, j) * (-0.5) ** j for j in range(m + 1)])
        c = np.convolve(p1, p2)
        beta = math.gamma(i + 1) * math.gamma(d + 1 - i) / math.gamma(d + 2)
        thetas.append(c / beta)
    return thetas


def build(tpb, stage=5):
    """Build the SPMD bass program. tpb = edge tiles per dst block."""
    nc = bacc.Bacc("TRN2", target_bir_lowering=False, debug=False,
                   num_devices=NCORES)

    # ---- DRAM I/O ----
    xT = nc.dram_tensor("xT", [FIN, PCP], bf16, kind="ExternalInput")
    w1 = nc.dram_tensor("w1", [FIN, H], bf16, kind="ExternalInput")
    w2 = nc.dram_tensor("w2", [H, H], bf16, kind="ExternalInput")
    c0 = nc.dram_tensor("c0", [H, H], f32, kind="ExternalInput")
    c1 = nc.dram_tensor("c1", [H, H], f32, kind="ExternalInput")
    c2 = nc.dram_tensor("c2", [H, H], f32, kind="ExternalInput")
    wm2 = nc.dram_tensor("wm2", [H, 2], bf16, kind="ExternalInput")
    b1 = nc.dram_tensor("b1", [P, 1], f32, kind="ExternalInput")
    b2 = nc.dram_tensor("b2", [P, 1], f32, kind="ExternalInput")
    bm1 = nc.dram_tensor("bm1", [P, 1], f32, kind="ExternalInput")
    bm2 = nc.dram_tensor("bm2", [2, 1], f32, kind="ExternalInput")
    iota_in = nc.dram_tensor("iota_in", [P, P], bf16, kind="ExternalInput")
    ident_in = nc.dram_tensor("ident_in", [P, P], f32, kind="ExternalInput")
    idx_g = nc.dram_tensor("idx_g", [NB, P, tpb], i32, kind="ExternalInput")
    cw = nc.dram_tensor("cw", [NB, P, tpb], bf16, kind="ExternalInput")
    wv = nc.dram_tensor("wv", [NB, P, tpb], f32, kind="ExternalInput")
    outT = nc.dram_tensor("outT", [2, PCP], f32, kind="ExternalOutput")

    # internal DRAM
    h_local = nc.dram_tensor("h_local", [PCP, H], bf16)
    h_full = nc.dram_tensor("h_full", [NPAD, H], bf16)
    l_local = nc.dram_tensor("l_local", [PCP, H], bf16)
    l_full = nc.dram_tensor("l_full", [NPAD, H], bf16)

    chunk_cols = [min(CH, PCP - c * CH) for c in range(NCH)]
    N_WT = 15   # weight/constant DMAs

    with ExitStack() as ctx:
        ent = ctx.enter_context
        # ---- SBUF ----
        hT = ent(nc.sbuf_tensor("hT", [P, PCP], f32))        # h; later z_acc
        lhT = ent(nc.sbuf_tensor("lhT", [P, PCP], f32))
        l2hT = ent(nc.sbuf_tensor("l2hT", [P, PCP], f32))
        xt_sb = ent(nc.sbuf_tensor("xt_sb", [P, 2, 4, CH], bf16))
        h1_sb = ent(nc.sbuf_tensor("h1_sb", [P, 2, CH], bf16))
        w1sb = ent(nc.sbuf_tensor("w1sb", [P, 4, H], bf16))
        w2sb = ent(nc.sbuf_tensor("w2sb", [P, H], bf16))
        c0sb = ent(nc.sbuf_tensor("c0sb", [P, H], f32))
        c1sb = ent(nc.sbuf_tensor("c1sb", [P, H], f32))
        c2sb = ent(nc.sbuf_tensor("c2sb", [P, H], f32))
        wm2sb = ent(nc.sbuf_tensor("wm2sb", [P, 2], bf16))
        b1sb = ent(nc.sbuf_tensor("b1sb", [P, 1], f32))
        b2sb = ent(nc.sbuf_tensor("b2sb", [P, 1], f32))
        bm1sb = ent(nc.sbuf_tensor("bm1sb", [P, 1], f32))
        bm2sb = ent(nc.sbuf_tensor("bm2sb", [2, 1], f32))
        iota_sb = ent(nc.sbuf_tensor("iota_sb", [P, P], bf16))
        ident_sb = ent(nc.sbuf_tensor("ident_sb", [P, P], f32))
        g_sb = ent(nc.sbuf_tensor("g_sb", [P, 2, tpb, H], bf16))
        s_sb = ent(nc.sbuf_tensor("s_sb", [P, 2, tpb, H], bf16))
        sel_sb = ent(nc.sbuf_tensor("sel_sb", [P, 2, P], bf16))
        cw_sb = ent(nc.sbuf_tensor("cw_sb", [P, 2, tpb], bf16))
        wv_sb = ent(nc.sbuf_tensor("wv_sb", [P, 2, tpb], f32))
        idx_sb = ent(nc.sbuf_tensor("idx_sb", [P, 2, tpb], i32))
        nm_sb = ent(nc.sbuf_tensor("nm_sb", [P, 2, H], bf16))
        zt_sb = ent(nc.sbuf_tensor("zt_sb", [P, 2, CH], f32))
        zb_sb = ent(nc.sbuf_tensor("zb_sb", [P, 2, CH], bf16))
        ot_sb = ent(nc.sbuf_tensor("ot_sb", [2, 2, CH], f32))

        # PSUM: 8 banks of [128, 512] f32; three double-buffered tensors
        ps_a = ent(nc.psum_tensor("ps_a", [P, 2, CH], f32))  # mm1 / z passes
        ps_b = ent(nc.psum_tensor("ps_b", [P, 2, CH], f32))  # mm2 / aggs / out
        ps_t = ent(nc.psum_tensor("ps_t", [P, 2, CH], f32))  # transposes (bank-padded)

        # semaphores
        s_wt = ent(nc.semaphore("s_wt"))
        s_x = ent(nc.semaphore("s_x"))
        s_idx = ent(nc.semaphore("s_idx"))
        s_hl = ent(nc.semaphore("s_hl"))
        s_ga = ent(nc.semaphore("s_ga"))
        s_gb = ent(nc.semaphore("s_gb"))
        s_g2a = ent(nc.semaphore("s_g2a"))
        s_g2b = ent(nc.semaphore("s_g2b"))
        s_out = ent(nc.semaphore("s_out"))
        cc_sem = ent(nc.semaphore("cc_sem"))
        pe1 = ent(nc.semaphore("pe1"))
        pe2 = ent(nc.semaphore("pe2"))
        petr = ent(nc.semaphore("petr"))
        pemm = ent(nc.semaphore("pemm"))
        pez = ent(nc.semaphore("pez"))
        peo = ent(nc.semaphore("peo"))
        act1 = ent(nc.semaphore("act1"))
        act2 = ent(nc.semaphore("act2"))
        actz = ent(nc.semaphore("actz"))
        arelu = ent(nc.semaphore("arelu"))
        abias = ent(nc.semaphore("abias"))
        dsub = ent(nc.semaphore("dsub"))
        dnm = ent(nc.semaphore("dnm"))
        dsel = ent(nc.semaphore("dsel"))
        dzadd = ent(nc.semaphore("dzadd"))
        block = ent(nc.Block())

        # ---------------- SYNC: HWDGE loads/stores ----------------
        @block.sync
        def _(sp):
            nwt = 0
            for k in range(4):
                sp.wait_ge(s_wt, 16 * nwt)
                sp.dma_start(w1sb[:, k], w1.ap()[k * P:(k + 1) * P, :]).then_inc(s_wt, 16)
                nwt += 1
            for dst_t, src_t in [(w2sb, w2), (c0sb, c0), (c1sb, c1), (c2sb, c2),
                                 (wm2sb, wm2), (b1sb, b1), (b2sb, b2),
                                 (bm1sb, bm1), (bm2sb, bm2),
                                 (iota_sb, iota_in), (ident_sb, ident_in)]:
                sp.wait_ge(s_wt, 16 * nwt)
                sp.dma_start(dst_t[:], src_t.ap()[:, :]).then_inc(s_wt, 16)
                nwt += 1

            # phase 1: x loads + h_local stores (store lags one chunk)
            nmi = 0
            for c in range(NCH):
                ncols = chunk_cols[c]
                if c >= 2:
                    sp.wait_ge(pe1, c - 1)
                if c >= 1:
                    sp.wait_ge(s_x, 64 * c)
                for k in range(4):
                    sp.dma_start(
                        xt_sb[:, c % 2, k, :ncols],
                        xT.ap()[k * P:(k + 1) * P, c * CH:c * CH + ncols],
                    ).then_inc(s_x, 16)
                if c >= 3:
                    for j in range(chunk_cols[c - 3] // P):
                        blk = 4 * (c - 3) + j
                        nmi += 1
                        sp.wait_ge(dnm, nmi)
                        sp.wait_ge(s_hl, 16 * (nmi - 1))
                        sp.dma_start(
                            h_local.ap()[blk * P:(blk + 1) * P, :],
                            nm_sb[:, (nmi - 1) % 2],
                        ).then_inc(s_hl, 16)
            for cc2 in range(NCH - 3, NCH):
                for j in range(chunk_cols[cc2] // P):
                    blk = 4 * cc2 + j
                    nmi += 1
                    sp.wait_ge(dnm, nmi)
                    sp.wait_ge(s_hl, 16 * (nmi - 1))
                    sp.dma_start(
                        h_local.ap()[blk * P:(blk + 1) * P, :],
                        nm_sb[:, (nmi - 1) % 2],
                    ).then_inc(s_hl, 16)
            assert nmi == NB

            # hop1: idx/cw loads + l_local stores (lag 2)
            for b in range(NB if stage >= 3 else 0):
                if b >= 2:
                    sp.wait_ge(pemm, b - 1)
                if b >= 1:
                    sp.wait_ge(s_idx, 48 * b)
                sp.dma_start(idx_sb[:, b % 2], idx_g.ap()[b]).then_inc(s_idx, 16)
                sp.dma_start(cw_sb[:, b % 2], cw.ap()[b]).then_inc(s_idx, 16)
                sp.dma_start(wv_sb[:, b % 2], wv.ap()[b]).then_inc(s_idx, 16)
                if b >= 2:
                    bb = b - 2
                    sp.wait_ge(dnm, NB + bb + 1)
                    sp.wait_ge(s_hl, 16 * (NB + bb))
                    sp.dma_start(
                        l_local.ap()[bb * P:(bb + 1) * P, :],
                        nm_sb[:, bb % 2],
                    ).then_inc(s_hl, 16)
            for bb in range(NB - 2 if stage >= 3 else NB, NB):
                sp.wait_ge(dnm, NB + bb + 1)
                sp.wait_ge(s_hl, 16 * (NB + bb))
                sp.dma_start(
                    l_local.ap()[bb * P:(bb + 1) * P, :],
                    nm_sb[:, bb % 2],
                ).then_inc(s_hl, 16)

            # hop2: idx/cw loads
            for b in range(NB if stage >= 4 else 0):
                if b >= 2:
                    sp.wait_ge(pemm, NB + b - 1)
                sp.wait_ge(s_idx, 48 * (NB + b))
                sp.dma_start(idx_sb[:, b % 2], idx_g.ap()[b]).then_inc(s_idx, 16)
                sp.dma_start(cw_sb[:, b % 2], cw.ap()[b]).then_inc(s_idx, 16)
                sp.dma_start(wv_sb[:, b % 2], wv.ap()[b]).then_inc(s_idx, 16)

            # head: outT stores  (actz counts: z1 NCH, relu NCH, bias NCH)
            for c in range(NCH if stage >= 5 else 0):
                ncols = chunk_cols[c]
                sp.wait_ge(abias, c + 1)
                sp.wait_ge(s_out, 16 * c)
                sp.dma_start(
                    outT.ap()[:, c * CH:c * CH + ncols],
                    ot_sb[:, c % 2, :ncols],
                ).then_inc(s_out, 16)

        # ---------------- GPSIMD: collectives + gathers ----------------
        @block.gpsimd
        def _(pl: bass.BassGpSimd):
            if stage < 2:
                return
            pl.wait_ge(s_hl, 16 * NB)
            pl.collective_compute(
                "AllGather", ALU.bypass,
                replica_groups=[list(range(NCORES))],
                ins=[h_local.ap().opt()],
                outs=[h_full.ap().opt()],
            ).then_inc(cc_sem, 1)
            pl.wait_ge(cc_sem, 1)
            for b in range(NB if stage >= 3 else 0):
                pl.wait_ge(s_idx, 48 * (b + 1))
                if b >= 2:
                    pl.wait_ge(pemm, b - 1)
                sgp = s_ga if b % 2 == 0 else s_gb
                if b >= 2:
                    pl.wait_ge(sgp, 16 * tpb * (b // 2))
                for t in range(tpb):
                    pl.indirect_dma_start(
                        out=g_sb[:, b % 2, t], out_offset=None,
                        in_=h_full.ap()[:, :],
                        in_offset=bass.IndirectOffsetOnAxis(
                            ap=idx_sb[:, b % 2, t:t + 1], axis=0),
                    ).then_inc(sgp, 16)
            if stage < 4:
                return
            pl.wait_ge(s_hl, 16 * 2 * NB)
            pl.collective_compute(
                "AllGather", ALU.bypass,
                replica_groups=[list(range(NCORES))],
                ins=[l_local.ap().opt()],
                outs=[l_full.ap().opt()],
            ).then_inc(cc_sem, 1)
            pl.wait_ge(cc_sem, 2)
            for b in range(NB):
                pl.wait_ge(s_idx, 48 * NB + 48 * (b + 1))
                if b >= 2:
                    pl.wait_ge(pemm, NB + b - 1)
                sgp = s_g2a if b % 2 == 0 else s_g2b
                if b >= 2:
                    pl.wait_ge(sgp, 16 * tpb * (b // 2))
                for t in range(tpb):
                    pl.indirect_dma_start(
                        out=g_sb[:, b % 2, t], out_offset=None,
                        in_=l_full.ap()[:, :],
                        in_offset=bass.IndirectOffsetOnAxis(
                            ap=idx_sb[:, b % 2, t:t + 1], axis=0),
                    ).then_inc(sgp, 16)

        # ---------------- TENSOR (PE) ----------------
        @block.tensor
        def _(pe: bass.BassTensorEngine):
            pe.wait_ge(s_wt, 16 * N_WT)

            def mm1(c):
                ncols = chunk_cols[c]
                pe.wait_ge(s_x, 64 * (c + 1))
                if c >= 2:
                    pe.wait_ge(act1, c - 1)
                mm = None
                for k in range(4):
                    mm = pe.matmul(ps_a[:, c % 2, :ncols], lhsT=w1sb[:, k],
                                   rhs=xt_sb[:, c % 2, k, :ncols],
                                   start=(k == 0), stop=(k == 3))
                mm.then_inc(pe1, 1)

            def mm2(c):
                ncols = chunk_cols[c]
                pe.wait_ge(act1, c + 1)
                if c >= 2:
                    pe.wait_ge(act2, c - 1)
                pe.matmul(ps_b[:, c % 2, :ncols], lhsT=w2sb[:],
                          rhs=h1_sb[:, c % 2, :ncols],
                          start=True, stop=True).then_inc(pe2, 1)

            tr_n = [0]

            def tr_phase1(c):
                pe.wait_ge(act2, c + 1)
                for j in range(chunk_cols[c] // P):
                    blk = 4 * c + j
                    i = tr_n[0]
                    if i >= 2:
                        pe.wait_ge(dnm, i - 1)
                    pe.transpose(ps_t[:, i % 2, :P], hT[:, blk * P:(blk + 1) * P],
                                 ident_sb[:]).then_inc(petr, 1)
                    tr_n[0] += 1

            for c in range(NCH + 2):
                if c < NCH:
                    mm1(c)
                if 1 <= c <= NCH:
                    mm2(c - 1)
                if 2 <= c:
                    tr_phase1(c - 2)
            assert tr_n[0] == NB

            def hop_mms(hop, b):
                gi = hop * NB + b
                if hop == 0:
                    sgp = s_ga if b % 2 == 0 else s_gb
                else:
                    sgp = s_g2a if b % 2 == 0 else s_g2b
                pe.wait_ge(sgp, 16 * tpb * (b // 2 + 1))
                pe.wait_ge(dsel, 2 * tpb * (gi + 1))
                if b >= 2:
                    pe.wait_ge(dsub, gi - 1)
                elif hop == 0:
                    pe.wait_ge(act2, NCH)      # ps_b handoff from phase 1
                else:
                    pe.wait_ge(dsub, NB)       # ps_b handoff from hop 1
                mm = None
                for t in range(tpb):
                    mm = pe.matmul(ps_b[:, b % 2, :P],
                                   lhsT=g_sb[:, b % 2, t],
                                   rhs=s_sb[:, b % 2, t],
                                   start=(t == 0), stop=(t == tpb - 1))
                mm.then_inc(pemm, 1)

            def tr_hop1(b):
                pe.wait_ge(dsub, b + 1)
                i = NB + b
                pe.wait_ge(dnm, i - 1)
                pe.transpose(ps_t[:, i % 2, :P], lhT[:, b * P:(b + 1) * P],
                             ident_sb[:]).then_inc(petr, 1)

            if stage < 3:
                return
            for b in range(NB + 1):
                if b < NB:
                    hop_mms(0, b)
                if b >= 1:
                    tr_hop1(b - 1)

            if stage < 4:
                return
            # z1: ps_a = C0^T @ hT chunks (hop1 consumed hT)
            pe.wait_ge(dsub, NB)
            for c in range(NCH):
                ncols = chunk_cols[c]
                if c >= 2:
                    pe.wait_ge(actz, c - 1)
                else:
                    pe.wait_ge(act1, NCH)      # ps_a handoff from phase 1
                pe.matmul(ps_a[:, c % 2, :ncols], lhsT=c0sb[:],
                          rhs=hT[:, c * CH:c * CH + ncols],
                          start=True, stop=True).then_inc(pez, 1)

            for b in range(NB):
                hop_mms(1, b)
            if stage < 5:
                return

            # z2: ps_a = C1^T @ lhT
            for c in range(NCH):
                ncols = chunk_cols[c]
                if c >= 2:
                    pe.wait_ge(dzadd, c - 1)
                else:
                    pe.wait_ge(actz, NCH)      # z1 fully consumed ps_a
                pe.matmul(ps_a[:, c % 2, :ncols], lhsT=c1sb[:],
                          rhs=lhT[:, c * CH:c * CH + ncols],
                          start=True, stop=True).then_inc(pez, 1)

            # z3 + head
            pe.wait_ge(dsub, 2 * NB)
            def z3mm(c):
                ncols = chunk_cols[c]
                if c >= 2:
                    pe.wait_ge(dzadd, NCH + c - 1)
                else:
                    pe.wait_ge(dzadd, NCH)
                pe.matmul(ps_a[:, c % 2, :ncols], lhsT=c2sb[:],
                          rhs=l2hT[:, c * CH:c * CH + ncols],
                          start=True, stop=True).then_inc(pez, 1)

            def outmm(c):
                ncols = chunk_cols[c]
                pe.wait_ge(arelu, c + 1)           # zb ready
                if c >= 2:
                    pe.wait_ge(abias, c - 1)       # ot consumed -> ps_b free
                else:
                    pe.wait_ge(dsub, 2 * NB)       # hop2 aggs consumed ps_b
                pe.matmul(ps_b[0:2, c % 2, :ncols], lhsT=wm2sb[:],
                          rhs=zb_sb[:, c % 2, :ncols],
                          start=True, stop=True).then_inc(peo, 1)

            for c in range(NCH):
                z3mm(c)
                if c >= 1:
                    outmm(c - 1)
            outmm(NCH - 1)

        # ---------------- VECTOR (DVE) ----------------
        @block.vector
        def _(dv: bass.BassVectorEngine):
            dv.wait_ge(s_wt, 16 * N_WT)
            # phase 1 node-major casts
            for i in range(NB):
                dv.wait_ge(petr, i + 1)
                if i >= 2:
                    dv.wait_ge(s_hl, 16 * (i - 1))
                dv.tensor_copy(out=nm_sb[:, i % 2], in_=ps_t[:, i % 2, :P]) \
                    .then_inc(dnm, 1)

            def hop_dve(hop, b):
                gi = hop * NB + b
                dv.wait_ge(s_idx, 48 * gi + 48)
                src = hT if hop == 0 else lhT
                dst = lhT if hop == 0 else l2hT
                for t in range(tpb):
                    dv.tensor_tensor(
                        out=sel_sb[:, t % 2],
                        in0=cw_sb[:, b % 2, t:t + 1].to_broadcast([P, P]),
                        in1=iota_sb[:],
                        op=ALU.is_equal,
                    ).then_inc(dsel, 1)
                    dv.drain()
                    dv.tensor_scalar(
                        out=s_sb[:, b % 2, t], in0=sel_sb[:, t % 2],
                        scalar1=wv_sb[:, b % 2, t:t + 1],
                        scalar2=None, op0=ALU.mult,
                    ).then_inc(dsel, 1)
                dv.wait_ge(pemm, gi + 1)
                dv.tensor_tensor(
                    out=dst[:, b * P:(b + 1) * P],
                    in0=src[:, b * P:(b + 1) * P],
                    in1=ps_b[:, b % 2, :P],
                    op=ALU.subtract,
                ).then_inc(dsub, 1)

            def nm_hop1(b):
                dv.wait_ge(petr, NB + b + 1)
                dv.wait_ge(s_hl, 16 * (NB + b - 1))
                dv.tensor_copy(out=nm_sb[:, b % 2], in_=ps_t[:, (NB + b) % 2, :P]) \
                    .then_inc(dnm, 1)

            if stage < 3:
                return
            for b in range(NB):
                hop_dve(0, b)
                if b >= 1:
                    nm_hop1(b - 1)
            nm_hop1(NB - 1)

            if stage < 4:
                return
            for b in range(NB):
                hop_dve(1, b)
            if stage < 5:
                return

            # z2 accumulate into z_acc (= hT region)
            for c in range(NCH):
                ncols = chunk_cols[c]
                dv.wait_ge(pez, NCH + c + 1)
                dv.wait_ge(actz, c + 1)        # z1 wrote hT chunk c
                dv.tensor_tensor(
                    out=hT[:, c * CH:c * CH + ncols],
                    in0=hT[:, c * CH:c * CH + ncols],
                    in1=ps_a[:, c % 2, :ncols],
                    op=ALU.add,
                ).then_inc(dzadd, 1)
            dv.drain()
            # z3 accumulate into zt tiles
            for c in range(NCH):
                ncols = chunk_cols[c]
                dv.wait_ge(pez, 2 * NCH + c + 1)
                if c >= 2:
                    dv.wait_ge(arelu, c - 1)        # zt slot consumed by relu
                dv.tensor_tensor(
                    out=zt_sb[:, c % 2, :ncols],
                    in0=hT[:, c * CH:c * CH + ncols],
                    in1=ps_a[:, c % 2, :ncols],
                    op=ALU.add,
                ).then_inc(dzadd, 1)

        # ---------------- SCALAR (ACT) ----------------
        @block.scalar
        def _(ac: bass.BassScalarEngine):
            ac.wait_ge(s_wt, 16 * N_WT)
            for c in range(NCH):
                ncols = chunk_cols[c]
                ac.wait_ge(pe1, c + 1)
                if c >= 2:
                    ac.wait_ge(pe2, c - 1)     # h1 slot consumed by mm2
                ac.activation(h1_sb[:, c % 2, :ncols], ps_a[:, c % 2, :ncols],
                              AF.Relu, bias=b1sb[:, 0:1], scale=1.0) \
                    .then_inc(act1, 1)
                ac.wait_ge(pe2, c + 1)
                ac.activation(hT[:, c * CH:c * CH + ncols],
                              ps_b[:, c % 2, :ncols],
                              AF.Relu, bias=b2sb[:, 0:1], scale=1.0) \
                    .then_inc(act2, 1)
            # z1: z_acc = ps_a + bm1 (in place over hT)
            if stage < 4:
                return
            for c in range(NCH):
                ncols = chunk_cols[c]
                ac.wait_ge(pez, c + 1)
                ac.activation(hT[:, c * CH:c * CH + ncols],
                              ps_a[:, c % 2, :ncols],
                              AF.Identity, bias=bm1sb[:, 0:1], scale=1.0) \
                    .then_inc(actz, 1)
            # z3 relu -> bf16, interleaved with out-bias
            def relu_c(c):
                ncols = chunk_cols[c]
                ac.wait_ge(dzadd, NCH + c + 1)
                if c >= 2:
                    ac.wait_ge(peo, c - 1)     # zb slot consumed by out mm
                ac.activation(zb_sb[:, c % 2, :ncols], zt_sb[:, c % 2, :ncols],
                              AF.Relu, bias=0.0, scale=1.0) \
                    .then_inc(arelu, 1)

            def bias_c(c):
                ncols = chunk_cols[c]
                ac.wait_ge(peo, c + 1)
                if c >= 2:
                    ac.wait_ge(s_out, 16 * (c - 1))
                ac.activation(ot_sb[:, c % 2, :ncols], ps_b[0:2, c % 2, :ncols],
                              AF.Identity, bias=bm2sb[:, 0:1], scale=1.0) \
                    .then_inc(abias, 1)

            if stage < 5:
                return
            for c in range(NCH):
                relu_c(c)
                if c >= 1:
                    bias_c(c - 1)
            bias_c(NCH - 1)

    nc.compile()
    return nc


def host_prep(x, src, dst, W1, b1, W2, b2, Wm1, bm1, Wm2, bm2):
    """Shard + preprocess inputs. Returns (in_maps, tpb)."""
    src = np.asarray(src).astype(np.int64)
    dst = np.asarray(dst).astype(np.int64)
    x = np.asarray(x, dtype=np.float32)

    thetas = _calculate_theta(D_POLY)
    C = [sum(float(thetas[i][k]) * np.asarray(Wm1, np.float64)[i * H:(i + 1) * H]
             for i in range(D_POLY + 1)).astype(np.float32)
         for k in range(D_POLY + 1)]
    C = [np.ascontiguousarray(ck) for ck in C]

    deg = np.bincount(dst, minlength=N_NODES)
    dinv = np.clip(deg, 1, None).astype(np.float64) ** -0.5
    w_edge = (dinv[dst] * dinv[src]).astype(np.float32)

    core_of = dst // PC
    loc = dst - core_of * PC
    blk = loc // P
    col = (loc % P).astype(np.float32)
    gsrc = ((src // PC) * PCP + (src % PC)).astype(np.int32)

    flat = core_of * NB + blk
    order = np.argsort(flat, kind="stable")
    counts = np.bincount(flat, minlength=NCORES * NB)
    cum = np.concatenate([[0], np.cumsum(counts)])
    tpb = int(np.ceil(counts.max() / P))

    iota_np = np.tile(
        np.arange(P, dtype=np.float32).astype(ml_dtypes.bfloat16)[None, :],
        (P, 1))
    ident_np = np.eye(P, dtype=np.float32)

    w1_b = np.ascontiguousarray(np.asarray(W1, np.float32)).astype(ml_dtypes.bfloat16)
    w2_b = np.ascontiguousarray(np.asarray(W2, np.float32)).astype(ml_dtypes.bfloat16)
    wm2_b = np.ascontiguousarray(np.asarray(Wm2, np.float32)).astype(ml_dtypes.bfloat16)

    in_maps = []
    for c in range(NCORES):
        xc = np.zeros((PCP, FIN), np.float32)
        xc[:PC] = x[c * PC:(c + 1) * PC]
        xT_c = np.ascontiguousarray(xc.T).astype(ml_dtypes.bfloat16)

        idx_gc = np.zeros((NB, P, tpb), np.int32)
        cw_c = np.zeros((NB, P, tpb), np.float32)
        wv_c = np.zeros((NB, P, tpb), np.float32)
        for b in range(NB):
            fi = c * NB + b
            es = order[cum[fi]:cum[fi + 1]]
            n = len(es)
            sl_idx = np.zeros(tpb * P, np.int32)
            sl_col = np.zeros(tpb * P, np.float32)
            sl_w = np.zeros(tpb * P, np.float32)
            sl_idx[:n] = gsrc[es]
            sl_col[:n] = col[es]
            sl_w[:n] = w_edge[es]
            idx_gc[b] = sl_idx.reshape(tpb, P).T
            cw_c[b] = sl_col.reshape(tpb, P).T
            wv_c[b] = sl_w.reshape(tpb, P).T

        in_maps.append(dict(
            xT=np.asarray(xT_c),
            w1=np.asarray(w1_b), w2=np.asarray(w2_b),
            c0=C[0], c1=C[1], c2=C[2],
            wm2=np.asarray(wm2_b),
            b1=np.ascontiguousarray(np.asarray(b1, np.float32).reshape(P, 1)),
            b2=np.ascontiguousarray(np.asarray(b2, np.float32).reshape(P, 1)),
            bm1=np.ascontiguousarray(np.asarray(bm1, np.float32).reshape(P, 1)),
            bm2=np.ascontiguousarray(np.asarray(bm2, np.float32).reshape(2, 1)),
            iota_in=np.asarray(iota_np),
            ident_in=ident_np,
            idx_g=idx_gc,
            cw=np.asarray(cw_c.astype(ml_dtypes.bfloat16)),
            wv=wv_c,
        ))
    return in_maps, tpb


_BUILD_CACHE = {}


def kernel(x, src, dst, W1, b1, W2, b2, Wm1, bm1, Wm2, bm2, _trace=False):
    in_maps, tpb = host_prep(x, src, dst, W1, b1, W2, b2, Wm1, bm1, Wm2, bm2)
    if tpb not in _BUILD_CACHE:
        _BUILD_CACHE[tpb] = build(tpb)
    nc = _BUILD_CACHE[tpb]
    r = run_bass_kernel_spmd(nc, in_maps, list(range(NCORES)), trace=_trace)
    out = np.zeros((N_NODES, 2), np.float32)
    for c in range(NCORES):
        out[c * PC:(c + 1) * PC] = np.asarray(r.results[c]["outT"]).T[:PC]
    kernel._last_results = r
    return out


# revision 15
# speedup vs baseline: 1.2245x; 1.0603x over previous
"""BWGNN (beta-wavelet GNN) forward on 8 TRN2 NeuronCores.

Sharding: nodes row-sharded 12500/core (padded to 12544 = 98 blocks of 128);
edges partitioned by dst so scatter-adds are local; per hop the bf16 feature
table is AllGathered and src rows are fetched by indirect DMA; the
segment-sum is a one-hot matmul into PSUM per 128-node dst block. Dense
weights replicated; the beta-wavelet polynomial is folded into the MLP-head
weights so only h, Lh, L^2h are needed.

kernel(**inputs) -> np.ndarray [100000, 2] float32
"""

import math
from contextlib import ExitStack

import numpy as np
import ml_dtypes

import concourse.bacc as bacc
import concourse.bass as bass
import concourse.mybir as mybir
from concourse.bass_utils import run_bass_kernel_spmd

# ---- problem constants (hardcoded) ----
N_NODES = 100000
N_EDGES = 1600000
FIN = 512
H = 128
D_POLY = 2
NCORES = 8
P = 128
PC = N_NODES // NCORES            # 12500
NB = (PC + P - 1) // P            # 98
PCP = NB * P                      # 12544
NPAD = NCORES * PCP               # 100352
CH = 512                          # node cols per dense chunk
NCH = (PCP + CH - 1) // CH        # 25 (last chunk 256)

f32 = mybir.dt.float32
bf16 = mybir.dt.bfloat16
i32 = mybir.dt.int32

AF = mybir.ActivationFunctionType
ALU = mybir.AluOpType


def _calculate_theta(d):
    thetas = []
    for i in range(d + 1):
        p1 = np.zeros(i + 1)
        p1[i] = 0.5 ** i
        m = d - i
        p2 = np.array([math.comb(m, j) * (-0.5) ** j for j in range(m + 1)])
        c = np.convolve(p1, p2)
        beta = math.gamma(i + 1) * math.gamma(d + 1 - i) / math.gamma(d + 2)
        thetas.append(c / beta)
    return thetas


def build(tpbs, stage=5):
    tpb = max(tpbs)
    cums = [0]
    for t_ in tpbs:
        cums.append(cums[-1] + t_)
    TOT = cums[-1]
    # per-parity cumulative gather-tile counts
    cum_par = {0: [0], 1: [0]}
    for b_, t_ in enumerate(tpbs):
        p_ = b_ % 2
        cum_par[p_].append(cum_par[p_][-1] + t_)
    par_idx = [0, 0]
    par_pos = []
    cnt = [0, 0]
    for b_ in range(len(tpbs)):
        p_ = b_ % 2
        cnt[p_] += 1
        par_pos.append(cnt[p_])  # 1-based position of block b within its parity
    """Build the SPMD bass program. tpbs[b] = edge tiles for dst block b."""
    nc = bacc.Bacc("TRN2", target_bir_lowering=False, debug=False,
                   num_devices=NCORES)

    # ---- DRAM I/O ----
    xT = nc.dram_tensor("xT", [FIN, PCP], bf16, kind="ExternalInput")
    w1 = nc.dram_tensor("w1", [FIN, H], bf16, kind="ExternalInput")
    w2 = nc.dram_tensor("w2", [H, H], bf16, kind="ExternalInput")
    c0 = nc.dram_tensor("c0", [H, H], f32, kind="ExternalInput")
    c1 = nc.dram_tensor("c1", [H, H], f32, kind="ExternalInput")
    c2 = nc.dram_tensor("c2", [H, H], f32, kind="ExternalInput")
    wm2 = nc.dram_tensor("wm2", [H, 2], bf16, kind="ExternalInput")
    b1 = nc.dram_tensor("b1", [P, 1], f32, kind="ExternalInput")
    b2 = nc.dram_tensor("b2", [P, 1], f32, kind="ExternalInput")
    bm1 = nc.dram_tensor("bm1", [P, 1], f32, kind="ExternalInput")
    bm2 = nc.dram_tensor("bm2", [2, 1], f32, kind="ExternalInput")
    iota_in = nc.dram_tensor("iota_in", [P, P], bf16, kind="ExternalInput")
    ident_in = nc.dram_tensor("ident_in", [P, P], f32, kind="ExternalInput")
    idx_g = nc.dram_tensor("idx_g", [NB, P, tpb], i32, kind="ExternalInput")
    cw = nc.dram_tensor("cw", [NB, P, tpb], bf16, kind="ExternalInput")
    wv = nc.dram_tensor("wv", [NB, P, tpb], f32, kind="ExternalInput")
    outT = nc.dram_tensor("outT", [2, PCP], f32, kind="ExternalOutput")

    # internal DRAM
    h_local = nc.dram_tensor("h_local", [PCP, H], bf16)
    h_full = nc.dram_tensor("h_full", [NPAD, H], bf16)
    l_local = nc.dram_tensor("l_local", [PCP, H], bf16)
    l_full = nc.dram_tensor("l_full", [NPAD, H], bf16)

    chunk_cols = [min(CH, PCP - c * CH) for c in range(NCH)]
    N_WT = 15   # weight/constant DMAs

    with ExitStack() as ctx:
        ent = ctx.enter_context
        # ---- SBUF ----
        hT = ent(nc.sbuf_tensor("hT", [P, PCP], f32))        # h; later z_acc
        lhT = ent(nc.sbuf_tensor("lhT", [P, PCP], f32))
        l2hT = ent(nc.sbuf_tensor("l2hT", [P, PCP], f32))
        xt_sb = ent(nc.sbuf_tensor("xt_sb", [P, 2, 4, CH], bf16))
        h1_sb = ent(nc.sbuf_tensor("h1_sb", [P, 2, CH], bf16))
        w1sb = ent(nc.sbuf_tensor("w1sb", [P, 4, H], bf16))
        w2sb = ent(nc.sbuf_tensor("w2sb", [P, H], bf16))
        c0sb = ent(nc.sbuf_tensor("c0sb", [P, H], f32))
        c1sb = ent(nc.sbuf_tensor("c1sb", [P, H], f32))
        c2sb = ent(nc.sbuf_tensor("c2sb", [P, H], f32))
        wm2sb = ent(nc.sbuf_tensor("wm2sb", [P, 2], bf16))
        b1sb = ent(nc.sbuf_tensor("b1sb", [P, 1], f32))
        b2sb = ent(nc.sbuf_tensor("b2sb", [P, 1], f32))
        bm1sb = ent(nc.sbuf_tensor("bm1sb", [P, 1], f32))
        bm2sb = ent(nc.sbuf_tensor("bm2sb", [2, 1], f32))
        iota_sb = ent(nc.sbuf_tensor("iota_sb", [P, P], bf16))
        ident_sb = ent(nc.sbuf_tensor("ident_sb", [P, P], f32))
        g_sb = ent(nc.sbuf_tensor("g_sb", [P, 2, tpb, H], bf16))
        s_sb = ent(nc.sbuf_tensor("s_sb", [P, 2, tpb, H], bf16))
        sel_sb = ent(nc.sbuf_tensor("sel_sb", [P, 2, P], bf16))
        cw_sb = ent(nc.sbuf_tensor("cw_sb", [P, 2, tpb], bf16))
        wv_sb = ent(nc.sbuf_tensor("wv_sb", [P, 2, tpb], f32))
        idx_sb = ent(nc.sbuf_tensor("idx_sb", [P, 2, tpb], i32))
        nm_sb = ent(nc.sbuf_tensor("nm_sb", [P, 2, H], bf16))
        zt_sb = ent(nc.sbuf_tensor("zt_sb", [P, 2, CH], f32))
        zb_sb = ent(nc.sbuf_tensor("zb_sb", [P, 2, CH], bf16))
        ot_sb = ent(nc.sbuf_tensor("ot_sb", [2, 2, CH], f32))

        # PSUM: 8 banks of [128, 512] f32; three double-buffered tensors
        ps_a = ent(nc.psum_tensor("ps_a", [P, 2, CH], f32))  # mm1 / z passes
        ps_b = ent(nc.psum_tensor("ps_b", [P, 2, CH], f32))  # mm2 / aggs / out
        ps_t = ent(nc.psum_tensor("ps_t", [P, 2, CH], f32))  # transposes (bank-padded)

        # semaphores
        s_wt = ent(nc.semaphore("s_wt"))
        s_x = ent(nc.semaphore("s_x"))
        s_idx = ent(nc.semaphore("s_idx"))
        s_hl = ent(nc.semaphore("s_hl"))
        s_ga = ent(nc.semaphore("s_ga"))
        s_gb = ent(nc.semaphore("s_gb"))
        s_g2a = ent(nc.semaphore("s_g2a"))
        s_g2b = ent(nc.semaphore("s_g2b"))
        s_out = ent(nc.semaphore("s_out"))
        cc_sem = ent(nc.semaphore("cc_sem"))
        pe1 = ent(nc.semaphore("pe1"))
        pe2 = ent(nc.semaphore("pe2"))
        petr = ent(nc.semaphore("petr"))
        pemm = ent(nc.semaphore("pemm"))
        pez = ent(nc.semaphore("pez"))
        peo = ent(nc.semaphore("peo"))
        act1 = ent(nc.semaphore("act1"))
        act2 = ent(nc.semaphore("act2"))
        actz = ent(nc.semaphore("actz"))
        arelu = ent(nc.semaphore("arelu"))
        abias = ent(nc.semaphore("abias"))
        dsub = ent(nc.semaphore("dsub"))
        dnm = ent(nc.semaphore("dnm"))
        dsel = ent(nc.semaphore("dsel"))
        dzadd = ent(nc.semaphore("dzadd"))
        block = ent(nc.Block())

        # ---------------- SYNC: HWDGE loads/stores ----------------
        @block.sync
        def _(sp):
            nwt = 0
            for k in range(4):
                sp.wait_ge(s_wt, 16 * nwt)
                sp.dma_start(w1sb[:, k], w1.ap()[k * P:(k + 1) * P, :]).then_inc(s_wt, 16)
                nwt += 1
            for dst_t, src_t in [(w2sb, w2), (c0sb, c0), (c1sb, c1), (c2sb, c2),
                                 (wm2sb, wm2), (b1sb, b1), (b2sb, b2),
                                 (bm1sb, bm1), (bm2sb, bm2),
                                 (iota_sb, iota_in), (ident_sb, ident_in)]:
                sp.wait_ge(s_wt, 16 * nwt)
                sp.dma_start(dst_t[:], src_t.ap()[:, :]).then_inc(s_wt, 16)
                nwt += 1

            # phase 1: x loads + h_local stores (store lags one chunk)
            nmi = 0
            for c in range(NCH):
                ncols = chunk_cols[c]
                if c >= 2:
                    sp.wait_ge(pe1, c - 1)
                if c >= 1:
                    sp.wait_ge(s_x, 64 * c)
                for k in range(4):
                    sp.dma_start(
                        xt_sb[:, c % 2, k, :ncols],
                        xT.ap()[k * P:(k + 1) * P, c * CH:c * CH + ncols],
                    ).then_inc(s_x, 16)
                if c >= 3:
                    for j in range(chunk_cols[c - 3] // P):
                        blk = 4 * (c - 3) + j
                        nmi += 1
                        sp.wait_ge(dnm, nmi)
                        sp.wait_ge(s_hl, 16 * (nmi - 1))
                        sp.dma_start(
                            h_local.ap()[blk * P:(blk + 1) * P, :],
                            nm_sb[:, (nmi - 1) % 2],
                        ).then_inc(s_hl, 16)
            for cc2 in range(NCH - 3, NCH):
                for j in range(chunk_cols[cc2] // P):
                    blk = 4 * cc2 + j
                    nmi += 1
                    sp.wait_ge(dnm, nmi)
                    sp.wait_ge(s_hl, 16 * (nmi - 1))
                    sp.dma_start(
                        h_local.ap()[blk * P:(blk + 1) * P, :],
                        nm_sb[:, (nmi - 1) % 2],
                    ).then_inc(s_hl, 16)
            assert nmi == NB

            # hop1: idx/cw loads + l_local stores (lag 2)
            for b in range(NB if stage >= 3 else 0):
                if b >= 2:
                    sp.wait_ge(pemm, b - 1)
                if b >= 1:
                    sp.wait_ge(s_idx, 48 * b)
                sp.dma_start(idx_sb[:, b % 2], idx_g.ap()[b]).then_inc(s_idx, 16)
                sp.dma_start(cw_sb[:, b % 2], cw.ap()[b]).then_inc(s_idx, 16)
                sp.dma_start(wv_sb[:, b % 2], wv.ap()[b]).then_inc(s_idx, 16)
                if b >= 2:
                    bb = b - 2
                    sp.wait_ge(dnm, NB + bb + 1)
                    sp.wait_ge(s_hl, 16 * (NB + bb))
                    sp.dma_start(
                        l_local.ap()[bb * P:(bb + 1) * P, :],
                        nm_sb[:, bb % 2],
                    ).then_inc(s_hl, 16)
            for bb in range(NB - 2 if stage >= 3 else NB, NB):
                sp.wait_ge(dnm, NB + bb + 1)
                sp.wait_ge(s_hl, 16 * (NB + bb))
                sp.dma_start(
                    l_local.ap()[bb * P:(bb + 1) * P, :],
                    nm_sb[:, bb % 2],
                ).then_inc(s_hl, 16)

            # hop2: idx/cw loads
            for b in range(NB if stage >= 4 else 0):
                if b >= 2:
                    sp.wait_ge(pemm, NB + b - 1)
                sp.wait_ge(s_idx, 48 * (NB + b))
                sp.dma_start(idx_sb[:, b % 2], idx_g.ap()[b]).then_inc(s_idx, 16)
                sp.dma_start(cw_sb[:, b % 2], cw.ap()[b]).then_inc(s_idx, 16)
                sp.dma_start(wv_sb[:, b % 2], wv.ap()[b]).then_inc(s_idx, 16)

            # head: outT stores  (actz counts: z1 NCH, relu NCH, bias NCH)
            for c in range(NCH if stage >= 5 else 0):
                ncols = chunk_cols[c]
                sp.wait_ge(abias, c + 1)
                sp.wait_ge(s_out, 16 * c)
                sp.dma_start(
                    outT.ap()[:, c * CH:c * CH + ncols],
                    ot_sb[:, c % 2, :ncols],
                ).then_inc(s_out, 16)

        # ---------------- GPSIMD: collectives + gathers ----------------
        @block.gpsimd
        def _(pl: bass.BassGpSimd):
            if stage < 2:
                return
            pl.wait_ge(s_hl, 16 * NB)
            pl.collective_compute(
                "AllGather", ALU.bypass,
                replica_groups=[list(range(NCORES))],
                ins=[h_local.ap().opt()],
                outs=[h_full.ap().opt()],
            ).then_inc(cc_sem, 1)
            pl.wait_ge(cc_sem, 1)
            for b in range(NB if stage >= 3 else 0):
                pl.wait_ge(s_idx, 48 * (b + 1))
                if b >= 2:
                    pl.wait_ge(pemm, b - 1)
                sgp = s_ga if b % 2 == 0 else s_gb
                if b >= 2:
                    pl.wait_ge(sgp, 16 * cum_par[b % 2][par_pos[b] - 1])
                for t in range(tpbs[b]):
                    pl.indirect_dma_start(
                        out=g_sb[:, b % 2, t], out_offset=None,
                        in_=h_full.ap()[:, :],
                        in_offset=bass.IndirectOffsetOnAxis(
                            ap=idx_sb[:, b % 2, t:t + 1], axis=0),
                    ).then_inc(sgp, 16)
            if stage < 4:
                return
            pl.wait_ge(s_hl, 16 * 2 * NB)
            pl.collective_compute(
                "AllGather", ALU.bypass,
                replica_groups=[list(range(NCORES))],
                ins=[l_local.ap().opt()],
                outs=[l_full.ap().opt()],
            ).then_inc(cc_sem, 1)
            pl.wait_ge(cc_sem, 2)
            for b in range(NB):
                pl.wait_ge(s_idx, 48 * NB + 48 * (b + 1))
                if b >= 2:
                    pl.wait_ge(pemm, NB + b - 1)
                sgp = s_g2a if b % 2 == 0 else s_g2b
                if b >= 2:
                    pl.wait_ge(sgp, 16 * cum_par[b % 2][par_pos[b] - 1])
                for t in range(tpbs[b]):
                    pl.indirect_dma_start(
                        out=g_sb[:, b % 2, t], out_offset=None,
                        in_=l_full.ap()[:, :],
                        in_offset=bass.IndirectOffsetOnAxis(
                            ap=idx_sb[:, b % 2, t:t + 1], axis=0),
                    ).then_inc(sgp, 16)

        # ---------------- TENSOR (PE) ----------------
        @block.tensor
        def _(pe: bass.BassTensorEngine):
            pe.wait_ge(s_wt, 16 * N_WT)

            def mm1(c):
                ncols = chunk_cols[c]
                pe.wait_ge(s_x, 64 * (c + 1))
                if c >= 2:
                    pe.wait_ge(act1, c - 1)
                mm = None
                for k in range(4):
                    mm = pe.matmul(ps_a[:, c % 2, :ncols], lhsT=w1sb[:, k],
                                   rhs=xt_sb[:, c % 2, k, :ncols],
                                   start=(k == 0), stop=(k == 3))
                mm.then_inc(pe1, 1)

            def mm2(c):
                ncols = chunk_cols[c]
                pe.wait_ge(act1, c + 1)
                if c >= 2:
                    pe.wait_ge(act2, c - 1)
                pe.matmul(ps_b[:, c % 2, :ncols], lhsT=w2sb[:],
                          rhs=h1_sb[:, c % 2, :ncols],
                          start=True, stop=True).then_inc(pe2, 1)

            tr_n = [0]

            def tr_phase1(c):
                pe.wait_ge(act2, c + 1)
                for j in range(chunk_cols[c] // P):
                    blk = 4 * c + j
                    i = tr_n[0]
                    if i >= 2:
                        pe.wait_ge(dnm, i - 1)
                    pe.transpose(ps_t[:, i % 2, :P], hT[:, blk * P:(blk + 1) * P],
                                 ident_sb[:]).then_inc(petr, 1)
                    tr_n[0] += 1

            for c in range(NCH + 2):
                if c < NCH:
                    mm1(c)
                if 1 <= c <= NCH:
                    mm2(c - 1)
                if 2 <= c:
                    tr_phase1(c - 2)
            assert tr_n[0] == NB

            def hop_mms(hop, b):
                gi = hop * NB + b
                if hop == 0:
                    sgp = s_ga if b % 2 == 0 else s_gb
                else:
                    sgp = s_g2a if b % 2 == 0 else s_g2b
                pe.wait_ge(sgp, 16 * cum_par[b % 2][par_pos[b]])
                pe.wait_ge(dsel, 2 * (hop * TOT + cums[b + 1]))
                if b >= 2:
                    pe.wait_ge(dsub, gi - 1)
                elif hop == 0:
                    pe.wait_ge(act2, NCH)      # ps_b handoff from phase 1
                else:
                    pe.wait_ge(dsub, NB)       # ps_b handoff from hop 1
                mm = None
                for t in range(tpbs[b]):
                    mm = pe.matmul(ps_b[:, b % 2, :P],
                                   lhsT=g_sb[:, b % 2, t],
                                   rhs=s_sb[:, b % 2, t],
                                   start=(t == 0), stop=(t == tpbs[b] - 1))
                mm.then_inc(pemm, 1)

            def tr_hop1(b):
                pe.wait_ge(dsub, b + 1)
                i = NB + b
                pe.wait_ge(dnm, i - 1)
                pe.transpose(ps_t[:, i % 2, :P], lhT[:, b * P:(b + 1) * P],
                             ident_sb[:]).then_inc(petr, 1)

            if stage < 3:
                return
            for b in range(NB + 1):
                if b < NB:
                    hop_mms(0, b)
                if b >= 1:
                    tr_hop1(b - 1)

            if stage < 4:
                return
            # z1: ps_a = C0^T @ hT chunks (hop1 consumed hT)
            pe.wait_ge(dsub, NB)
            for c in range(NCH):
                ncols = chunk_cols[c]
                if c >= 2:
                    pe.wait_ge(actz, c - 1)
                else:
                    pe.wait_ge(act1, NCH)      # ps_a handoff from phase 1
                pe.matmul(ps_a[:, c % 2, :ncols], lhsT=c0sb[:],
                          rhs=hT[:, c * CH:c * CH + ncols],
                          start=True, stop=True).then_inc(pez, 1)

            for b in range(NB):
                hop_mms(1, b)
            if stage < 5:
                return

            # z2: ps_a = C1^T @ lhT
            for c in range(NCH):
                ncols = chunk_cols[c]
                if c >= 2:
                    pe.wait_ge(dzadd, c - 1)
                else:
                    pe.wait_ge(actz, NCH)      # z1 fully consumed ps_a
                pe.matmul(ps_a[:, c % 2, :ncols], lhsT=c1sb[:],
                          rhs=lhT[:, c * CH:c * CH + ncols],
                          start=True, stop=True).then_inc(pez, 1)

            # z3 + head
            pe.wait_ge(dsub, 2 * NB)
            def z3mm(c):
                ncols = chunk_cols[c]
                if c >= 2:
                    pe.wait_ge(dzadd, NCH + c - 1)
                else:
                    pe.wait_ge(dzadd, NCH)
                pe.matmul(ps_a[:, c % 2, :ncols], lhsT=c2sb[:],
                          rhs=l2hT[:, c * CH:c * CH + ncols],
                          start=True, stop=True).then_inc(pez, 1)

            def outmm(c):
                ncols = chunk_cols[c]
                pe.wait_ge(arelu, c + 1)           # zb ready
                if c >= 2:
                    pe.wait_ge(abias, c - 1)       # ot consumed -> ps_b free
                else:
                    pe.wait_ge(dsub, 2 * NB)       # hop2 aggs consumed ps_b
                pe.matmul(ps_b[0:2, c % 2, :ncols], lhsT=wm2sb[:],
                          rhs=zb_sb[:, c % 2, :ncols],
                          start=True, stop=True).then_inc(peo, 1)

            for c in range(NCH):
                z3mm(c)
                if c >= 1:
                    outmm(c - 1)
            outmm(NCH - 1)

        # ---------------- VECTOR (DVE) ----------------
        @block.vector
        def _(dv: bass.BassVectorEngine):
            dv.wait_ge(s_wt, 16 * N_WT)
            # phase 1 node-major casts
            for i in range(NB):
                dv.wait_ge(petr, i + 1)
                if i >= 2:
                    dv.wait_ge(s_hl, 16 * (i - 1))
                dv.tensor_copy(out=nm_sb[:, i % 2], in_=ps_t[:, i % 2, :P]) \
                    .then_inc(dnm, 1)

            def hop_dve(hop, b):
                gi = hop * NB + b
                dv.wait_ge(s_idx, 48 * gi + 48)
                src = hT if hop == 0 else lhT
                dst = lhT if hop == 0 else l2hT
                for t in range(tpbs[b]):
                    dv.tensor_tensor(
                        out=sel_sb[:, t % 2],
                        in0=cw_sb[:, b % 2, t:t + 1].to_broadcast([P, P]),
                        in1=iota_sb[:],
                        op=ALU.is_equal,
                    ).then_inc(dsel, 1)
                    dv.drain()
                    dv.tensor_scalar(
                        out=s_sb[:, b % 2, t], in0=sel_sb[:, t % 2],
                        scalar1=wv_sb[:, b % 2, t:t + 1],
                        scalar2=None, op0=ALU.mult,
                    ).then_inc(dsel, 1)
                dv.wait_ge(pemm, gi + 1)
                dv.tensor_tensor(
                    out=dst[:, b * P:(b + 1) * P],
                    in0=src[:, b * P:(b + 1) * P],
                    in1=ps_b[:, b % 2, :P],
                    op=ALU.subtract,
                ).then_inc(dsub, 1)

            def nm_hop1(b):
                dv.wait_ge(petr, NB + b + 1)
                dv.wait_ge(s_hl, 16 * (NB + b - 1))
                dv.tensor_copy(out=nm_sb[:, b % 2], in_=ps_t[:, (NB + b) % 2, :P]) \
                    .then_inc(dnm, 1)

            if stage < 3:
                return
            for b in range(NB):
                hop_dve(0, b)
                if b >= 1:
                    nm_hop1(b - 1)
            nm_hop1(NB - 1)

            if stage < 4:
                return
            for b in range(NB):
                hop_dve(1, b)
            if stage < 5:
                return

            # z2 accumulate into z_acc (= hT region)
            for c in range(NCH):
                ncols = chunk_cols[c]
                dv.wait_ge(pez, NCH + c + 1)
                dv.wait_ge(actz, c + 1)        # z1 wrote hT chunk c
                dv.tensor_tensor(
                    out=hT[:, c * CH:c * CH + ncols],
                    in0=hT[:, c * CH:c * CH + ncols],
                    in1=ps_a[:, c % 2, :ncols],
                    op=ALU.add,
                ).then_inc(dzadd, 1)
            dv.drain()
            # z3 accumulate into zt tiles
            for c in range(NCH):
                ncols = chunk_cols[c]
                dv.wait_ge(pez, 2 * NCH + c + 1)
                if c >= 2:
                    dv.wait_ge(arelu, c - 1)        # zt slot consumed by relu
                dv.tensor_tensor(
                    out=zt_sb[:, c % 2, :ncols],
                    in0=hT[:, c * CH:c * CH + ncols],
                    in1=ps_a[:, c % 2, :ncols],
                    op=ALU.add,
                ).then_inc(dzadd, 1)

        # ---------------- SCALAR (ACT) ----------------
        @block.scalar
        def _(ac: bass.BassScalarEngine):
            ac.wait_ge(s_wt, 16 * N_WT)
            for c in range(NCH):
                ncols = chunk_cols[c]
                ac.wait_ge(pe1, c + 1)
                if c >= 2:
                    ac.wait_ge(pe2, c - 1)     # h1 slot consumed by mm2
                ac.activation(h1_sb[:, c % 2, :ncols], ps_a[:, c % 2, :ncols],
                              AF.Relu, bias=b1sb[:, 0:1], scale=1.0) \
                    .then_inc(act1, 1)
                ac.wait_ge(pe2, c + 1)
                ac.activation(hT[:, c * CH:c * CH + ncols],
                              ps_b[:, c % 2, :ncols],
                              AF.Relu, bias=b2sb[:, 0:1], scale=1.0) \
                    .then_inc(act2, 1)
            # z1: z_acc = ps_a + bm1 (in place over hT)
            if stage < 4:
                return
            for c in range(NCH):
                ncols = chunk_cols[c]
                ac.wait_ge(pez, c + 1)
                ac.activation(hT[:, c * CH:c * CH + ncols],
                              ps_a[:, c % 2, :ncols],
                              AF.Identity, bias=bm1sb[:, 0:1], scale=1.0) \
                    .then_inc(actz, 1)
            # z3 relu -> bf16, interleaved with out-bias
            def relu_c(c):
                ncols = chunk_cols[c]
                ac.wait_ge(dzadd, NCH + c + 1)
                if c >= 2:
                    ac.wait_ge(peo, c - 1)     # zb slot consumed by out mm
                ac.activation(zb_sb[:, c % 2, :ncols], zt_sb[:, c % 2, :ncols],
                              AF.Relu, bias=0.0, scale=1.0) \
                    .then_inc(arelu, 1)

            def bias_c(c):
                ncols = chunk_cols[c]
                ac.wait_ge(peo, c + 1)
                if c >= 2:
                    ac.wait_ge(s_out, 16 * (c - 1))
                ac.activation(ot_sb[:, c % 2, :ncols], ps_b[0:2, c % 2, :ncols],
                              AF.Identity, bias=bm2sb[:, 0:1], scale=1.0) \
                    .then_inc(abias, 1)

            if stage < 5:
                return
            for c in range(NCH):
                relu_c(c)
                if c >= 1:
                    bias_c(c - 1)
            bias_c(NCH - 1)

    nc.compile()
    return nc


def host_prep(x, src, dst, W1, b1, W2, b2, Wm1, bm1, Wm2, bm2):
    """Shard + preprocess inputs. Returns (in_maps, tpb)."""
    src = np.asarray(src).astype(np.int64)
    dst = np.asarray(dst).astype(np.int64)
    x = np.asarray(x, dtype=np.float32)

    thetas = _calculate_theta(D_POLY)
    C = [sum(float(thetas[i][k]) * np.asarray(Wm1, np.float64)[i * H:(i + 1) * H]
             for i in range(D_POLY + 1)).astype(np.float32)
         for k in range(D_POLY + 1)]
    C = [np.ascontiguousarray(ck) for ck in C]

    deg = np.bincount(dst, minlength=N_NODES)
    dinv = np.clip(deg, 1, None).astype(np.float64) ** -0.5
    w_edge = (dinv[dst] * dinv[src]).astype(np.float32)

    core_of = dst // PC
    loc = dst - core_of * PC
    blk = loc // P
    col = (loc % P).astype(np.float32)
    gsrc = ((src // PC) * PCP + (src % PC)).astype(np.int32)

    flat = core_of * NB + blk
    order = np.argsort(flat, kind="stable")
    counts = np.bincount(flat, minlength=NCORES * NB)
    cum = np.concatenate([[0], np.cumsum(counts)])
    cmat = counts.reshape(NCORES, NB)
    tpbs = [max(1, int(np.ceil(cmat[:, b].max() / P))) for b in range(NB)]
    tpb = max(tpbs)

    iota_np = np.tile(
        np.arange(P, dtype=np.float32).astype(ml_dtypes.bfloat16)[None, :],
        (P, 1))
    ident_np = np.eye(P, dtype=np.float32)

    w1_b = np.ascontiguousarray(np.asarray(W1, np.float32)).astype(ml_dtypes.bfloat16)
    w2_b = np.ascontiguousarray(np.asarray(W2, np.float32)).astype(ml_dtypes.bfloat16)
    wm2_b = np.ascontiguousarray(np.asarray(Wm2, np.float32)).astype(ml_dtypes.bfloat16)

    in_maps = []
    for c in range(NCORES):
        xc = np.zeros((PCP, FIN), np.float32)
        xc[:PC] = x[c * PC:(c + 1) * PC]
        xT_c = np.ascontiguousarray(xc.T).astype(ml_dtypes.bfloat16)

        idx_gc = np.zeros((NB, P, tpb), np.int32)
        cw_c = np.zeros((NB, P, tpb), np.float32)
        wv_c = np.zeros((NB, P, tpb), np.float32)
        for b in range(NB):
            fi = c * NB + b
            es = order[cum[fi]:cum[fi + 1]]
            n = len(es)
            sl_idx = np.zeros(tpb * P, np.int32)
            sl_col = np.zeros(tpb * P, np.float32)
            sl_w = np.zeros(tpb * P, np.float32)
            sl_idx[:n] = gsrc[es]
            sl_col[:n] = col[es]
            sl_w[:n] = w_edge[es]
            idx_gc[b] = sl_idx.reshape(tpb, P).T
            cw_c[b] = sl_col.reshape(tpb, P).T
            wv_c[b] = sl_w.reshape(tpb, P).T

        in_maps.append(dict(
            xT=np.asarray(xT_c),
            w1=np.asarray(w1_b), w2=np.asarray(w2_b),
            c0=C[0], c1=C[1], c2=C[2],
            wm2=np.asarray(wm2_b),
            b1=np.ascontiguousarray(np.asarray(b1, np.float32).reshape(P, 1)),
            b2=np.ascontiguousarray(np.asarray(b2, np.float32).reshape(P, 1)),
            bm1=np.ascontiguousarray(np.asarray(bm1, np.float32).reshape(P, 1)),
            bm2=np.ascontiguousarray(np.asarray(bm2, np.float32).reshape(2, 1)),
            iota_in=np.asarray(iota_np),
            ident_in=ident_np,
            idx_g=idx_gc,
            cw=np.asarray(cw_c.astype(ml_dtypes.bfloat16)),
            wv=wv_c,
        ))
    return in_maps, tuple(tpbs)


_BUILD_CACHE = {}


def kernel(x, src, dst, W1, b1, W2, b2, Wm1, bm1, Wm2, bm2, _trace=False):
    in_maps, tpbs = host_prep(x, src, dst, W1, b1, W2, b2, Wm1, bm1, Wm2, bm2)
    if tpbs not in _BUILD_CACHE:
        _BUILD_CACHE[tpbs] = build(list(tpbs))
    nc = _BUILD_CACHE[tpbs]
    r = run_bass_kernel_spmd(nc, in_maps, list(range(NCORES)), trace=_trace)
    out = np.zeros((N_NODES, 2), np.float32)
    for c in range(NCORES):
        out[c * PC:(c + 1) * PC] = np.asarray(r.results[c]["outT"]).T[:PC]
    kernel._last_results = r
    return out
